# revision 5
# baseline (speedup 1.0000x reference)
"""Bahdanau-attention scoring kernel for 8 TRN2 NeuronCores (fp8 DoubleRow).

Reference computation (S=2048, B=32, H=1024):
    cat    = concat([broadcast(hidden), enc], axis=2)          # [S,B,2H]
    alphas = tanh(einsum('sbk,hk->sbh', cat, W_attn) + b_attn) # [S,B,H]
    scores = einsum('sbh,h->sb', alphas, v)                    # [S,B]
    out    = softmax(scores.T, axis=1)[:, None, :]             # [B,1,S]

Because hidden broadcasts over S, the concat-matmul splits into
    z[s,b,:] = W2ᵀ enc[s,b,:] + hp[b,:],   hp[b,:] = W1 hidden[b] + b_attn.
hp is a [B,H] constant (0.4% of the FLOPs) and is computed on host; only
the big S×B×H×H term runs on device.

The big matmul (2048·4·1024·1024 MACs per core) runs in fp8 e4m3 with
perf_mode=DoubleRow (2 fp8 MACs per PE cell per cycle).  Host pre-scales
enc×8 and W2×64 to keep values clear of the e4m3 subnormal range; the
1/512 descale and the hp bias-add ride on the ACT activation op:
    alq = tanh(z_psum * DESCALE + hp[:,ht,b])
one [P,512] ACT op per h'-tile, reading PSUM directly.

Engine split (per half-row of 512 s-positions):
  PE : 32 DoubleRow matmuls (the roofline term, ~216ns each) + one
       K=128 ones-matmul that reduces the weighted sum over partitions.
  ACT: 8 fused tanh ops (+ per-row exp).
  DVE: the v-weighted accumulation ws += v[h']·alq[h'] as an 8-step
       f32 scalar_tensor_tensor chain (final step rounds to bf16),
       plus the score-row copy.
This keeps ACT (~6.3us/half) and DVE (~5.6us/half) under the PE
(~7.1us/half) so the fp8 stream never waits on eviction.

Schedule notes:
  - ~9 dummy matmuls (ones x zeros) run during the initial DMA wait so
    the PE HAM clock-gate is already warm (2.4GHz) when real data lands.
  - w2/hp ride the scalar HWDGE queue, enc the sync HWDGE queue, both
    in priority order; the first enc chunk is a 128KB quarter-tile so
    the first matmul issues ~2.5us after triggers start.
  - finops (ones-matmul + score copy + exp) trail the fp8 stream
    through a pending-FIFO so the PE never waits on DVE/ACT.
  - output DMAs trigger on the sync queue (never stalls ACT).

Sharding: data-parallel over batch.  Core c handles batches 4c..4c+3.
"""

import sys

for _p in ("/opt/trn_rl_repo", "/root/.axon_site/_ro/trn_rl_repo"):
    if _p not in sys.path:
        sys.path.insert(0, _p)

import numpy as np
import ml_dtypes

import concourse.bass as bass  # noqa: F401  (bass must import before tile)
import concourse.mybir as mybir
import concourse.tile as tile
from concourse import bacc
from concourse.bass_utils import run_bass_kernel_spmd

S, B, H = 2048, 32, 1024
NCORES = 8
BL = B // NCORES          # batches per core (4)
P = 128                   # SBUF partitions
KT2 = H // P              # k-subtiles of 128 (8)
NKT = KT2 // 2            # DoubleRow k-pairs per z tile (4)
HT = H // P               # h'-tiles (8)

E_SCALE, W_SCALE = 8.0, 64.0
DESCALE = 1.0 / (E_SCALE * W_SCALE)

F8 = mybir.dt.float8e4
BF16 = mybir.dt.bfloat16
F32 = mybir.dt.float32
AFT = mybir.ActivationFunctionType
MUL = mybir.AluOpType.mult
ADD = mybir.AluOpType.add
DR = mybir.MatmulPerfMode.DoubleRow

SKEW = 3                  # pending-FIFO depth (finops trailing the fp8 stream)
NWARM = 9                 # dummy matmuls to warm the PE clock gate

_nc_cache = None


def build():
    nc = bacc.Bacc()
    enc = nc.declare_dram_parameter("enc", [BL, NKT, P, 2, S], F8,
                                    isOutput=False)
    w2 = nc.declare_dram_parameter("w2", [NKT, P, 2, H], F8, isOutput=False)
    hp = nc.declare_dram_parameter("hp", [P, HT, BL], F32, isOutput=False)
    vv = nc.declare_dram_parameter("v", [P, HT], F32, isOutput=False)
    out = nc.declare_dram_parameter("out", [BL, S], F32, isOutput=True)

    with tile.TileContext(nc) as tc:
        with (
            tc.tile_pool(name="const", bufs=1) as cpool,
            tc.tile_pool(name="encp", bufs=8) as encp,
            tc.tile_pool(name="alqp", bufs=8) as alqp,
            tc.tile_pool(name="wsp", bufs=3) as wsp,
            tc.tile_pool(name="smallp", bufs=2) as smallp,
            tc.tile_pool(name="zps", bufs=3, space="PSUM") as zps,
            tc.tile_pool(name="sps", bufs=2, space="PSUM") as sps,
        ):
            # --- PE warm-up: dummy matmuls on memset tiles keep the HAM
            # clock gate busy during the initial DMA wait so the real
            # stream starts at 2.4GHz ---
            ones = cpool.tile([P, 1], BF16)
            nc.vector.memset(ones[:], 1.0)
            dum = cpool.tile([P, 512], BF16)
            nc.vector.memset(dum[:], 0.0)
            warm_ps = sps.tile([P, 512], F32, tag="srow", name="warm")
            for i in range(NWARM):
                nc.tensor.matmul(warm_ps[0:1, :], ones[:], dum[:],
                                 start=True, stop=True)

            # --- constants.  w2/hp on the scalar HWDGE queue, enc on
            # the sync HWDGE queue, in strict priority order ---
            w2p = [cpool.tile([P, 2, H], F8, tag=f"w2_{kt}", name=f"w2_{kt}")
                   for kt in range(NKT)]
            nc.scalar.dma_start(w2p[0][:], w2[0])
            hp_sb = cpool.tile([P, HT, BL], F32)
            nc.scalar.dma_start(hp_sb[:], hp[:])
            for kt in range(1, NKT):
                nc.scalar.dma_start(w2p[kt][:], w2[kt])
            v_sb = cpool.tile([P, HT], F32)
            nc.scalar.dma_start(v_sb[:], vv[:])

            etp0 = [encp.tile([P, 2, S], F8, tag="enc", name=f"et0_{kt}")
                    for kt in range(NKT)]
            # first quarter-tile (128KB) lands first so the PE can start;
            # then kt half-tiles in ramp-wave order
            nc.sync.dma_start(etp0[0][:, :, 0:512], enc[0, 0][:, :, 0:512])
            nc.sync.dma_start(etp0[0][:, :, 512:S], enc[0, 0][:, :, 512:S])
            for kt in range(1, NKT):
                nc.sync.dma_start(etp0[kt][:, :, 0:S // 2],
                                  enc[0, kt][:, :, 0:S // 2])
            for kt in range(1, NKT):
                nc.sync.dma_start(etp0[kt][:, :, S // 2:S],
                                  enc[0, kt][:, :, S // 2:S])

            # --- main loop ---
            pending = []

            def drain(n):
                while len(pending) > n:
                    pending.pop(0)()

            for b in range(BL):
                scb = smallp.tile([1, S], F32, tag="sb", name=f"scb{b}")
                last_b = b == BL - 1
                ex = smallp.tile([1, S], F32, tag="ex", name=f"ex{b}")
                tots = []
                if b == 0:
                    etp = etp0
                else:
                    etp = [encp.tile([P, 2, S], F8, tag="enc",
                                     name=f"et{b}_{kt}")
                           for kt in range(NKT)]
                    for kt in range(NKT):
                        nc.sync.dma_start(etp[kt][:], enc[b, kt])
                for half in range(4):
                    row = half
                    first = b == 0 and half == 0
                    score_ps = sps.tile([P, 512], F32, tag="srow",
                                        name=f"srow{b}_{row}")
                    zw = []
                    if first:
                        # ramp wave: kt-outer across the first 3
                        # pair-groups so the PE consumes each kt-wave
                        # (w2[kt] + enc chunk) as it lands
                        zw = [zps.tile([P, 2, 512], F32, tag="z",
                                       name=f"zw{i}") for i in range(3)]
                        for kt in range(NKT):
                            for hq in range(3):
                                for ht2 in range(2):
                                    ht = hq * 2 + ht2
                                    nc.tensor.matmul(
                                        zw[hq][:, ht2, :],
                                        w2p[kt][:, :, ht * P:(ht + 1) * P],
                                        etp[kt][:, :, 0:512],
                                        start=(kt == 0),
                                        stop=(kt == NKT - 1),
                                        perf_mode=DR)
                    ws = None
                    wsf = None
                    for htp in range(HT // 2):
                        if first and htp < 3:
                            z_ps = zw[htp]
                        else:
                            z_ps = zps.tile([P, 2, 512], F32, tag="z")
                            for ht2 in range(2):
                                ht = htp * 2 + ht2
                                for kt in range(NKT):
                                    nc.tensor.matmul(
                                        z_ps[:, ht2, :],
                                        w2p[kt][:, :, ht * P:(ht + 1) * P],
                                        etp[kt][:, :,
                                                half * 512:(half + 1) * 512],
                                        start=(kt == 0),
                                        stop=(kt == NKT - 1),
                                        perf_mode=DR)
                        # z-eviction fused on ACT: alq = tanh(z/512 + hp)
                        # then the v-weighted partition accumulation on
                        # DVE: ws += v[h'] * alq   (f32 chain, final
                        # step rounds once to bf16)
                        alq = alqp.tile([P, 2, 512], BF16, tag="alq")
                        for ht2 in range(2):
                            ht = htp * 2 + ht2
                            nc.scalar.activation(
                                alq[:, ht2, :], z_ps[:, ht2, :], AFT.Tanh,
                                bias=hp_sb[:, ht, b:b + 1], scale=DESCALE)
                            if ht == 0:
                                ws = wsp.tile([P, 512], F32, tag="ws")
                                nc.vector.tensor_scalar_mul(
                                    ws[:], alq[:, ht2, :], v_sb[:, 0:1])
                            elif ht < HT - 1:
                                ws2 = wsp.tile([P, 512], F32, tag="ws")
                                nc.vector.scalar_tensor_tensor(
                                    ws2[:], alq[:, ht2, :],
                                    v_sb[:, ht:ht + 1], ws[:], MUL, ADD)
                                ws = ws2
                            else:
                                wsf = wsp.tile([P, 512], BF16, tag="wsf",
                                               bufs=6)
                                nc.vector.scalar_tensor_tensor(
                                    wsf[:], alq[:, ht2, :],
                                    v_sb[:, ht:ht + 1], ws[:], MUL, ADD)
                        drain(1 if b == BL - 1 else SKEW)

                    # score row: ones-matmul reduces ws over partitions,
                    # DVE copies it out of PSUM; per-row exp for the
                    # last batch keeps softmax off the tail
                    def finops(b=b, row=row, score_ps=score_ps, wsf=wsf,
                               scb=scb, ex=ex, tots=tots, last_b=last_b):
                        nc.tensor.matmul(score_ps[0:1, :], ones[:], wsf[:],
                                         start=True, stop=True)
                        nc.vector.tensor_copy(
                            scb[:, row * 512:(row + 1) * 512],
                            score_ps[0:1, :])
                        if last_b:
                            tr = smallp.tile([1, 1], F32, tag="tot",
                                             bufs=8, name=f"tr{b}_{row}")
                            nc.scalar.activation(
                                ex[:, row * 512:(row + 1) * 512],
                                scb[:, row * 512:(row + 1) * 512],
                                AFT.Exp, accum_out=tr[:])
                            tots.append(tr)
                    pending.append(finops)

                # softmax row b (no max-sub: |scores| <= sum|v| ~ 26)
                def softmax(b=b, scb=scb, ex=ex, tots=tots,
                            last_b=last_b):
                    if last_b:
                        t01 = smallp.tile([1, 1], F32, tag="t01", bufs=2)
                        nc.vector.tensor_add(t01[:], tots[0][:], tots[1][:])
                        t23 = smallp.tile([1, 1], F32, tag="t23", bufs=2)
                        nc.vector.tensor_add(t23[:], tots[2][:], tots[3][:])
                        tot = smallp.tile([1, 1], F32, tag="tot", bufs=8,
                                          name=f"tot{b}")
                        nc.vector.tensor_add(tot[:], t01[:], t23[:])
                    else:
                        tot = smallp.tile([1, 1], F32, tag="tot", bufs=8,
                                          name=f"tot{b}")
                        nc.scalar.activation(ex[:], scb[:], AFT.Exp,
                                             accum_out=tot[:])
                    rec = smallp.tile([1, 1], F32, tag="rec", bufs=2)
                    nc.vector.reciprocal(rec[:], tot[:])
                    osb = smallp.tile([1, S], F32, tag="osb")
                    if last_b:
                        # split the final normalize across DVE and ACT
                        # so the tail chain is ~1us instead of 2.1
                        nc.vector.tensor_scalar_mul(
                            osb[:, 0:1280], ex[:, 0:1280], rec[:, 0:1])
                        nc.scalar.activation(
                            osb[:, 1280:S], ex[:, 1280:S], AFT.Copy,
                            scale=rec[:, 0:1])
                        nc.sync.dma_start(out[b:b + 1, 0:1280],
                                          osb[:, 0:1280])
                        nc.sync.dma_start(out[b:b + 1, 1280:S],
                                          osb[:, 1280:S])
                    else:
                        nc.vector.tensor_scalar_mul(osb[:], ex[:],
                                                    rec[:, 0:1])
                        nc.sync.dma_start(out[b:b + 1, :], osb[:])
                pending.append(softmax)
            drain(0)
    nc.compile()
    return nc


def _get_nc():
    global _nc_cache
    if _nc_cache is None:
        _nc_cache = build()
    return _nc_cache


def kernel(hidden, encoder_outputs, W_attn, b_attn, v, _trace=False):
    f8 = ml_dtypes.float8_e4m3
    bf16 = ml_dtypes.bfloat16
    hidden = np.asarray(hidden, dtype=np.float32)
    encoder_outputs = np.asarray(encoder_outputs, dtype=np.float32)
    W_attn = np.asarray(W_attn, dtype=np.float32)
    b_attn = np.asarray(b_attn, dtype=np.float32)
    v = np.asarray(v, dtype=np.float32)

    # hp[b,h] = sum_k hidden[b,k] W_attn[h,k] + b_attn[h] — tiny
    # (0.4% of the FLOPs), computed on host
    hp_full = hidden[0] @ W_attn[:, :H].T + b_attn               # [B, H]
    w2 = np.ascontiguousarray(
        (W_attn[:, H:].T * W_SCALE).reshape(NKT, 2, P, H)
        .transpose(0, 2, 1, 3)).astype(f8)
    vv = np.ascontiguousarray(v.reshape(HT, P).T)               # [P, HT] f32
    # [B, NKT, P, 2, S]: per-(b, k-pair) tiles, one contiguous 4KB
    # per-partition segment each, pre-scaled fp8
    enc_t = (encoder_outputs.transpose(1, 2, 0) * E_SCALE).astype(f8)
    enc_t = np.ascontiguousarray(
        enc_t.reshape(B, NKT, 2, P, S).transpose(0, 1, 3, 2, 4))

    in_maps = []
    for c in range(NCORES):
        bsl = slice(c * BL, (c + 1) * BL)
        in_maps.append({
            "enc": np.ascontiguousarray(enc_t[bsl]),
            "w2": w2,
            "hp": np.ascontiguousarray(
                hp_full[bsl].reshape(BL, HT, P).transpose(2, 1, 0)),
            "v": vv,
        })

    nc = _get_nc()
    res = run_bass_kernel_spmd(
        nc, in_maps, core_ids=list(range(NCORES)), trace=_trace,
    )
    parts = [res.results[c]["out"] for c in range(NCORES)]      # [BL, S] each
    full = np.concatenate(parts, axis=0)
    out = full[:, None, :].astype(np.float32)                   # [B, 1, S]
    if _trace:
        return out, res
    return out


# revision 8
# speedup vs baseline: 1.0685x; 1.0685x over previous
"""Bahdanau-attention scoring kernel for 8 TRN2 NeuronCores (fp8 DoubleRow).

Reference computation (S=2048, B=32, H=1024):
    cat    = concat([broadcast(hidden), enc], axis=2)          # [S,B,2H]
    alphas = tanh(einsum('sbk,hk->sbh', cat, W_attn) + b_attn) # [S,B,H]
    scores = einsum('sbh,h->sb', alphas, v)                    # [S,B]
    out    = softmax(scores.T, axis=1)[:, None, :]             # [B,1,S]

Because hidden broadcasts over S, the concat-matmul splits into
    z[s,b,:] = W2ᵀ enc[s,b,:] + hp[b,:],   hp[b,:] = W1 hidden[b] + b_attn.
hp is a [B,H] constant (0.4% of the FLOPs) and is computed on host; only
the big S×B×H×H term runs on device.

The big matmul (2048·4·1024·1024 MACs per core) runs in fp8 e4m3 with
perf_mode=DoubleRow (2 fp8 MACs per PE cell per cycle).  Host pre-scales
enc×8 and W2×64 to keep values clear of the e4m3 subnormal range; the
1/512 descale and the hp bias-add ride on the ACT activation op:
    alq = tanh(z_psum * DESCALE + hp[:,ht,b])
one [P,512] ACT op per h'-tile, reading PSUM directly.

Engine split (per half-row of 512 s-positions):
  PE : 32 DoubleRow matmuls (the roofline term, ~216ns each) + one
       K=128 ones-matmul that reduces the weighted sum over partitions.
  ACT: 8 fused tanh ops (+ per-row exp).
  DVE: the v-weighted accumulation ws += v[h']·alq[h'] as an 8-step
       f32 scalar_tensor_tensor chain (final step rounds to bf16),
       plus the score-row copy.
This keeps ACT (~6.3us/half) and DVE (~5.6us/half) under the PE
(~7.1us/half) so the fp8 stream never waits on eviction.

Schedule notes:
  - ~9 dummy matmuls (ones x zeros) run during the initial DMA wait so
    the PE HAM clock-gate is already warm (2.4GHz) when real data lands.
  - w2/hp ride the scalar HWDGE queue, enc the sync HWDGE queue, both
    in priority order; the first enc chunk is a 128KB quarter-tile so
    the first matmul issues ~2.5us after triggers start.
  - finops (ones-matmul + score copy + exp) trail the fp8 stream
    through a pending-FIFO so the PE never waits on DVE/ACT.
  - output DMAs trigger on the sync queue (never stalls ACT).

Sharding: data-parallel over batch.  Core c handles batches 4c..4c+3.
"""

import sys

for _p in ("/opt/trn_rl_repo", "/root/.axon_site/_ro/trn_rl_repo"):
    if _p not in sys.path:
        sys.path.insert(0, _p)

import numpy as np
import ml_dtypes

import concourse.bass as bass  # noqa: F401  (bass must import before tile)
import concourse.mybir as mybir
import concourse.tile as tile
from concourse import bacc
from concourse.bass_utils import run_bass_kernel_spmd

S, B, H = 2048, 32, 1024
NCORES = 8
BL = B // NCORES          # batches per core (4)
P = 128                   # SBUF partitions
KT2 = H // P              # k-subtiles of 128 (8)
NKT = KT2 // 2            # DoubleRow k-pairs per z tile (4)
HT = H // P               # h'-tiles (8)

E_SCALE, W_SCALE = 8.0, 64.0
DESCALE = 1.0 / (E_SCALE * W_SCALE)

F8 = mybir.dt.float8e4
BF16 = mybir.dt.bfloat16
F32 = mybir.dt.float32
AFT = mybir.ActivationFunctionType
MUL = mybir.AluOpType.mult
ADD = mybir.AluOpType.add
DR = mybir.MatmulPerfMode.DoubleRow

SKEW = 3                  # pending-FIFO depth (finops trailing the fp8 stream)
NWARM = 5                 # dummy matmuls to warm the PE clock gate

_nc_cache = None


def build():
    nc = bacc.Bacc()
    enc = nc.declare_dram_parameter("enc", [BL, NKT, P, 2, S], F8,
                                    isOutput=False)
    w2 = nc.declare_dram_parameter("w2", [NKT, P, 2, H], F8, isOutput=False)
    hp = nc.declare_dram_parameter("hp", [P, HT, BL], F32, isOutput=False)
    vv = nc.declare_dram_parameter("v", [P, HT], F32, isOutput=False)
    vvb = nc.declare_dram_parameter("vb", [P, HT], BF16, isOutput=False)
    out = nc.declare_dram_parameter("out", [BL, S], F32, isOutput=True)

    with tile.TileContext(nc) as tc:
        with (
            tc.tile_pool(name="const", bufs=1) as cpool,
            tc.tile_pool(name="encp", bufs=8) as encp,
            tc.tile_pool(name="alqp", bufs=8) as alqp,
            tc.tile_pool(name="wsp", bufs=3) as wsp,
            tc.tile_pool(name="smallp", bufs=2) as smallp,
            tc.tile_pool(name="zps", bufs=3, space="PSUM") as zps,
            tc.tile_pool(name="sps", bufs=2, space="PSUM") as sps,
        ):
            # --- PE warm-up: dummy matmuls on memset tiles keep the HAM
            # clock gate busy during the initial DMA wait so the real
            # stream starts at 2.4GHz ---
            ones = cpool.tile([P, 1], BF16)
            nc.vector.memset(ones[:], 1.0)
            dum = cpool.tile([P, 512], BF16)
            nc.vector.memset(dum[:], 0.0)
            warm_ps = sps.tile([P, 512], F32, tag="srow", name="warm")
            for i in range(NWARM):
                nc.tensor.matmul(warm_ps[0:1, :], ones[:], dum[:],
                                 start=True, stop=True)

            # --- constants.  w2/hp on the scalar HWDGE queue, enc on
            # the sync HWDGE queue, in strict priority order ---
            w2p = [cpool.tile([P, 2, H], F8, tag=f"w2_{kt}", name=f"w2_{kt}")
                   for kt in range(NKT)]
            nc.scalar.dma_start(w2p[0][:], w2[0])
            hp_sb = cpool.tile([P, HT, BL], F32)
            nc.scalar.dma_start(hp_sb[:], hp[:])
            for kt in range(1, NKT):
                nc.scalar.dma_start(w2p[kt][:], w2[kt])
            v_sb = cpool.tile([P, HT], F32)
            nc.scalar.dma_start(v_sb[:], vv[:])
            vb_sb = cpool.tile([P, HT], BF16)
            nc.scalar.dma_start(vb_sb[:], vvb[:])

            etp0 = [encp.tile([P, 2, S], F8, tag="enc", name=f"et0_{kt}")
                    for kt in range(NKT)]
            # quarter-tiles (128KB) for every kt land first, rate-matched
            # to the PE ramp waves; the rests follow
            for kt in range(NKT):
                nc.sync.dma_start(etp0[kt][:, :, 0:512],
                                  enc[0, kt][:, :, 0:512])
            for kt in range(NKT):
                nc.sync.dma_start(etp0[kt][:, :, 512:S],
                                  enc[0, kt][:, :, 512:S])

            # --- main loop ---
            pending = []

            def drain(n):
                while len(pending) > n:
                    pending.pop(0)()

            for b in range(BL):
                scb = smallp.tile([1, S], F32, tag="sb", name=f"scb{b}")
                last_b = b == BL - 1
                ex = smallp.tile([1, S], F32, tag="ex", name=f"ex{b}")
                tots = []
                if b == 0:
                    etp = etp0
                else:
                    etp = [encp.tile([P, 2, S], F8, tag="enc",
                                     name=f"et{b}_{kt}")
                           for kt in range(NKT)]
                    for kt in range(NKT):
                        nc.sync.dma_start(etp[kt][:], enc[b, kt])
                for half in range(4):
                    row = half
                    first = b == 0 and half == 0
                    score_ps = sps.tile([P, 512], F32, tag="srow",
                                        name=f"srow{b}_{row}")
                    zw = []
                    if first:
                        # ramp wave: kt-outer across the first 3
                        # pair-groups so the PE consumes each kt-wave
                        # (w2[kt] + enc chunk) as it lands
                        zw = [zps.tile([P, 2, 512], F32, tag="z",
                                       name=f"zw{i}") for i in range(3)]
                        for kt in range(NKT):
                            for hq in range(3):
                                for ht2 in range(2):
                                    ht = hq * 2 + ht2
                                    nc.tensor.matmul(
                                        zw[hq][:, ht2, :],
                                        w2p[kt][:, :, ht * P:(ht + 1) * P],
                                        etp[kt][:, :, 0:512],
                                        start=(kt == 0),
                                        stop=(kt == NKT - 1),
                                        perf_mode=DR)
                    ws = None
                    wsf = None
                    quad = []
                    for htp in range(HT // 2):
                        if first and htp < 3:
                            z_ps = zw[htp]
                        else:
                            z_ps = zps.tile([P, 2, 512], F32, tag="z")
                            for ht2 in range(2):
                                ht = htp * 2 + ht2
                                for kt in range(NKT):
                                    nc.tensor.matmul(
                                        z_ps[:, ht2, :],
                                        w2p[kt][:, :, ht * P:(ht + 1) * P],
                                        etp[kt][:, :,
                                                half * 512:(half + 1) * 512],
                                        start=(kt == 0),
                                        stop=(kt == NKT - 1),
                                        perf_mode=DR)
                        # z-eviction fused on ACT: alq = tanh(z/512 + hp)
                        # then the v-contraction: for b<3 a DVE f32
                        # chain ws += v[h']*alq (hidden under the
                        # stream); for the last batch PE v-matmul quads
                        # (short tail — no DVE latency to drain)
                        alq = alqp.tile([P, 2, 512], BF16, tag="alq")
                        for ht2 in range(2):
                            ht = htp * 2 + ht2
                            nc.scalar.activation(
                                alq[:, ht2, :], z_ps[:, ht2, :], AFT.Tanh,
                                bias=hp_sb[:, ht, b:b + 1], scale=DESCALE)
                            if last_b:
                                continue
                            if ht == 0:
                                ws = wsp.tile([P, 512], F32, tag="ws")
                                nc.vector.tensor_scalar_mul(
                                    ws[:], alq[:, ht2, :], v_sb[:, 0:1])
                            elif ht < HT - 1:
                                ws2 = wsp.tile([P, 512], F32, tag="ws")
                                nc.vector.scalar_tensor_tensor(
                                    ws2[:], alq[:, ht2, :],
                                    v_sb[:, ht:ht + 1], ws[:], MUL, ADD)
                                ws = ws2
                            else:
                                wsf = wsp.tile([P, 512], BF16, tag="wsf",
                                               bufs=6)
                                nc.vector.scalar_tensor_tensor(
                                    wsf[:], alq[:, ht2, :],
                                    v_sb[:, ht:ht + 1], ws[:], MUL, ADD)
                        if last_b:
                            quad.append((htp, alq))
                            if htp % 2 == 1:
                                # 4 concurrent M=1 matmuls on PE column
                                # groups; slot j owns s-quarter j and
                                # accumulates all 8 h'-tiles
                                def vmms(quad=tuple(quad),
                                         score_ps=score_ps):
                                    for hq, a in quad:
                                        for ht2 in range(2):
                                            ht = hq * 2 + ht2
                                            for j in range(4):
                                                nc.tensor.matmul(
                                                    score_ps[32 * j:
                                                             32 * j + 1,
                                                             0:P],
                                                    vb_sb[:, ht:ht + 1],
                                                    a[:, ht2,
                                                      j * P:(j + 1) * P],
                                                    start=(ht == 0),
                                                    stop=(ht == HT - 1),
                                                    tile_position=(0, 32 * j))
                                pending.append(vmms)
                                quad = []
                        drain(1 if b == BL - 1 else SKEW)

                    # score row: for b<3 a ones-matmul reduces ws over
                    # partitions and DVE copies it out; for the last
                    # batch gather the 4 column-group slots + per-row
                    # exp so softmax stays off the tail
                    def finops(b=b, row=row, score_ps=score_ps, wsf=wsf,
                               scb=scb, ex=ex, tots=tots, last_b=last_b):
                        if last_b:
                            for j in range(4):
                                nc.vector.tensor_copy(
                                    scb[:, row * 512 + j * P:
                                        row * 512 + (j + 1) * P],
                                    score_ps[32 * j:32 * j + 1, 0:P])
                            tr = smallp.tile([1, 1], F32, tag="tot",
                                             bufs=8, name=f"tr{b}_{row}")
                            nc.scalar.activation(
                                ex[:, row * 512:(row + 1) * 512],
                                scb[:, row * 512:(row + 1) * 512],
                                AFT.Exp, accum_out=tr[:])
                            tots.append(tr)
                        else:
                            nc.tensor.matmul(score_ps[0:1, :], ones[:],
                                             wsf[:], start=True, stop=True)
                            nc.vector.tensor_copy(
                                scb[:, row * 512:(row + 1) * 512],
                                score_ps[0:1, :])
                    pending.append(finops)

                # softmax row b (no max-sub: |scores| <= sum|v| ~ 26)
                def softmax(b=b, scb=scb, ex=ex, tots=tots,
                            last_b=last_b):
                    if last_b:
                        t01 = smallp.tile([1, 1], F32, tag="t01", bufs=2)
                        nc.vector.tensor_add(t01[:], tots[0][:], tots[1][:])
                        t23 = smallp.tile([1, 1], F32, tag="t23", bufs=2)
                        nc.vector.tensor_add(t23[:], tots[2][:], tots[3][:])
                        tot = smallp.tile([1, 1], F32, tag="tot", bufs=8,
                                          name=f"tot{b}")
                        nc.vector.tensor_add(tot[:], t01[:], t23[:])
                    else:
                        tot = smallp.tile([1, 1], F32, tag="tot", bufs=8,
                                          name=f"tot{b}")
                        nc.scalar.activation(ex[:], scb[:], AFT.Exp,
                                             accum_out=tot[:])
                    rec = smallp.tile([1, 1], F32, tag="rec", bufs=2)
                    nc.vector.reciprocal(rec[:], tot[:])
                    osb = smallp.tile([1, S], F32, tag="osb")
                    if last_b:
                        # split the final normalize across DVE and ACT
                        # so the tail chain is ~1us instead of 2.1
                        nc.vector.tensor_scalar_mul(
                            osb[:, 0:1280], ex[:, 0:1280], rec[:, 0:1])
                        nc.scalar.activation(
                            osb[:, 1280:S], ex[:, 1280:S], AFT.Copy,
                            scale=rec[:, 0:1])
                        nc.sync.dma_start(out[b:b + 1, 0:1280],
                                          osb[:, 0:1280])
                        nc.sync.dma_start(out[b:b + 1, 1280:S],
                                          osb[:, 1280:S])
                    else:
                        nc.vector.tensor_scalar_mul(osb[:], ex[:],
                                                    rec[:, 0:1])
                        nc.sync.dma_start(out[b:b + 1, :], osb[:])
                pending.append(softmax)
            drain(0)
    nc.compile()
    return nc


def _get_nc():
    global _nc_cache
    if _nc_cache is None:
        _nc_cache = build()
    return _nc_cache


def kernel(hidden, encoder_outputs, W_attn, b_attn, v, _trace=False):
    f8 = ml_dtypes.float8_e4m3
    bf16 = ml_dtypes.bfloat16
    hidden = np.asarray(hidden, dtype=np.float32)
    encoder_outputs = np.asarray(encoder_outputs, dtype=np.float32)
    W_attn = np.asarray(W_attn, dtype=np.float32)
    b_attn = np.asarray(b_attn, dtype=np.float32)
    v = np.asarray(v, dtype=np.float32)

    # hp[b,h] = sum_k hidden[b,k] W_attn[h,k] + b_attn[h] — tiny
    # (0.4% of the FLOPs), computed on host
    hp_full = hidden[0] @ W_attn[:, :H].T + b_attn               # [B, H]
    w2 = np.ascontiguousarray(
        (W_attn[:, H:].T * W_SCALE).reshape(NKT, 2, P, H)
        .transpose(0, 2, 1, 3)).astype(f8)
    vv = np.ascontiguousarray(v.reshape(HT, P).T)               # [P, HT] f32
    # [B, NKT, P, 2, S]: per-(b, k-pair) tiles, one contiguous 4KB
    # per-partition segment each, pre-scaled fp8
    enc_t = (encoder_outputs.transpose(1, 2, 0) * E_SCALE).astype(f8)
    enc_t = np.ascontiguousarray(
        enc_t.reshape(B, NKT, 2, P, S).transpose(0, 1, 3, 2, 4))

    in_maps = []
    for c in range(NCORES):
        bsl = slice(c * BL, (c + 1) * BL)
        in_maps.append({
            "enc": np.ascontiguousarray(enc_t[bsl]),
            "w2": w2,
            "hp": np.ascontiguousarray(
                hp_full[bsl].reshape(BL, HT, P).transpose(2, 1, 0)),
            "v": vv,
            "vb": vv.astype(bf16),
        })

    nc = _get_nc()
    res = run_bass_kernel_spmd(
        nc, in_maps, core_ids=list(range(NCORES)), trace=_trace,
    )
    parts = [res.results[c]["out"] for c in range(NCORES)]      # [BL, S] each
    full = np.concatenate(parts, axis=0)
    out = full[:, None, :].astype(np.float32)                   # [B, 1, S]
    if _trace:
        return out, res
    return out
